# revision 5
# baseline (speedup 1.0000x reference)
"""ContrastiveLoss Trainium2 kernel (8 NeuronCores, SPMD row-sharded).

Math (reference):
    f = features / ||features||_row            (L2 normalize)
    s_ij = (f_i . f_j) / T,  T = 0.1
    Z_i = sum_{j != i} exp(s_ij)
    per_row_i = (num_pos_i * log(Z_i) - sum_j mask_ij s_ij) / (num_pos_i + eps)
    loss = mean(per_row)
where mask = same-label excluding self.  sum_j mask_ij s_ij = 10*(f_i . g_{label_i}) - 10
with g_c = sum_{j: label_j = c} f_j  (class sums) -- avoids any O(N^2) masked work.

Each core owns 1024 rows: computes its (1024 x 8192) similarity block in bf16 on
the PE, exponentiates on ACT with fused row-sum accumulation, and assembles its
per-row losses.  Host only shards/concatenates and takes the final mean.
"""

import numpy as np
import ml_dtypes

TEMP_INV = 10.0  # 1/temperature
EPS = 1e-8
N, D, NCORES = 8192, 512, 8
RPC = N // NCORES        # 1024 rows per core
RT = RPC // 128          # 8 row tiles (128 rows) per core
MT = N // RPC            # 8 column mega-tiles of 1024 rows
CG = 2048                # psum/exp column-group width
NCG = N // CG            # 4 column groups
KC = D // 128            # 4 contraction chunks

_prog_cache = None


def _build_program():
    import concourse.bacc as bacc
    import concourse.tile as tile
    import concourse.hw_specs as hw_specs
    from concourse import mybir

    # Pin every ACT function we use (Exp/Ln/Copy/Identity) to the single table
    # set that contains them all, so walrus never inserts a mid-kernel ~2.7us
    # table switch.  Mutates the functools.cache'd dict in place; indices into
    # act_info.json are preserved because only set *contents* change.
    tabs = hw_specs.get_activation_tables("gen3")
    keep = "natural_log_exp_and_others"
    if keep in tabs:
        for name in tabs:
            if name != keep:
                tabs[name] = set()

    f32, bf16 = mybir.dt.float32, mybir.dt.bfloat16
    A = mybir.ActivationFunctionType
    Alu = mybir.AluOpType
    X = mybir.AxisListType.X

    nc = bacc.Bacc("TRN2", target_bir_lowering=False, debug=False,
                   num_devices=NCORES)

    feat = nc.dram_tensor("feat", [N, D], bf16, kind="ExternalInput")
    xrow = nc.dram_tensor("xrow", [RPC, D], bf16, kind="ExternalInput")
    ohc = nc.dram_tensor("ohc", [N, 2], bf16, kind="ExternalInput")
    ohr = nc.dram_tensor("ohr", [128, RT, 2], f32, kind="ExternalInput")
    npos = nc.dram_tensor("npos", [128, RT], f32, kind="ExternalInput")
    invn = nc.dram_tensor("invn", [128, RT], f32, kind="ExternalInput")
    outp = nc.dram_tensor("out", [128, RT], f32, kind="ExternalOutput")

    featv = feat.ap().rearrange("(m g p) d -> m p g d", p=128, g=RPC // 128)
    xrowv = xrow.ap().rearrange("(g p) d -> p g d", p=128)
    ohcv = ohc.ap().rearrange("(t p) c -> p t c", p=128)

    from contextlib import ExitStack

    with tile.TileContext(nc) as tc, ExitStack() as ctx:
        singles = ctx.enter_context(tc.tile_pool(name="singles", bufs=1))
        xin = ctx.enter_context(tc.tile_pool(name="xin", bufs=3))
        bpool = ctx.enter_context(tc.tile_pool(name="bpool", bufs=3))
        scrp = ctx.enter_context(tc.tile_pool(name="scr", bufs=2))
        ssp = ctx.enter_context(tc.tile_pool(name="ss", bufs=6))
        expp = ctx.enter_context(tc.tile_pool(name="expscr", bufs=3))
        zp = ctx.enter_context(tc.tile_pool(name="zac", bufs=3))
        dramp = ctx.enter_context(tc.tile_pool(name="dram", bufs=1, space="DRAM"))

        # persistent transposed features: fT[c][cg] holds d-chunk c of columns
        # [cg*2048, (cg+1)*2048); fTr[c] the same for this core's own rows.
        fT = [[singles.tile([128, CG], bf16, tag=f"fT{c}_{g}", name=f"fT{c}_{g}")
               for g in range(NCG)] for c in range(KC)]
        fTr = [singles.tile([128, RPC], bf16, tag=f"fTr{c}", name=f"fTr{c}")
               for c in range(KC)]

        ohc_sb = singles.tile([128, N // 128, 2], bf16, tag="ohc")
        nc.sync.dma_start(out=ohc_sb, in_=ohcv)
        ohr_sb = singles.tile([128, RT, 2], f32, tag="ohr")
        nc.sync.dma_start(out=ohr_sb, in_=ohr.ap())
        npos_sb = singles.tile([128, RT], f32, tag="npos")
        nc.sync.dma_start(out=npos_sb, in_=npos.ap())
        invn_sb = singles.tile([128, RT], f32, tag="invn")
        nc.sync.dma_start(out=invn_sb, in_=invn.ap())

        diag_ss = singles.tile([128, RT], f32, tag="diag")
        ZE = singles.tile([128, RT], f32, tag="ZE")
        g_sb = singles.tile([2, D], bf16, tag="gsb")
        gT_sb = singles.tile([128, KC, 2], bf16, tag="gT")
        rd_sb = singles.tile([128, RT, 2], f32, tag="rd")

        def prep_megatile(src_ap, own):
            """Load 1024 rows, normalize, cast to bf16; returns B tile."""
            x = xin.tile([128, RPC // 128, D], bf16, tag="xin")
            nc.sync.dma_start(out=x, in_=src_ap)
            ss = ssp.tile([128, RPC // 128], f32, tag="ss")
            scr = scrp.tile([128, RPC // 128, D], bf16, tag="scr")
            for g in range(RPC // 128):
                nc.vector.tensor_tensor(out=scr[:, g], in0=x[:, g],
                                        in1=x[:, g], op=Alu.mult)
                nc.vector.tensor_reduce(out=ss[:, g:g + 1], in_=scr[:, g],
                                        axis=X, op=Alu.add)
            lnb = ssp.tile([128, RPC // 128], f32, tag="lnb")
            nc.scalar.activation(out=lnb, in_=ss, func=A.Ln)
            rn = ssp.tile([128, RPC // 128], f32, tag="rn")
            nc.scalar.activation(out=rn, in_=lnb, func=A.Exp, scale=-0.5)
            b = bpool.tile([128, RPC // 128, D], bf16, tag="b")
            for g in range(RPC // 128):
                nc.vector.tensor_scalar(
                    out=b[:, g], in0=x[:, g], scalar1=rn[:, g:g + 1],
                    scalar2=None, op0=Alu.mult)
            return b

        with tc.tile_pool(name="gps", bufs=1, space="PSUM") as gpp:
            g_ps = gpp.tile([2, D], f32)

            # own rows first (feeds the matmul lhsT)
            b = prep_megatile(xrowv, own=True)
            scrf = scrp.tile([128, RT, D], f32, tag="scrf")
            for g in range(RT):
                nc.vector.tensor_tensor(out=scrf[:, g], in0=b[:, g],
                                        in1=b[:, g], op=Alu.mult)
                nc.vector.tensor_reduce(out=diag_ss[:, g:g + 1],
                                        in_=scrf[:, g], axis=X, op=Alu.add)
                for c in range(KC):
                    nc.sync.dma_start_transpose(
                        out=fTr[c][:, 128 * g:128 * g + 128],
                        in_=b[:, g, 128 * c:128 * c + 128])

            # all column mega-tiles: transpose into fT and accumulate class sums
            for m in range(MT):
                b = prep_megatile(featv[m], own=False)
                for g in range(RPC // 128):
                    t = m * (RPC // 128) + g
                    nc.tensor.matmul(g_ps, lhsT=ohc_sb[:, t], rhs=b[:, g],
                                     start=(t == 0), stop=(t == N // 128 - 1))
                    cg, off = t // 16, 128 * (t % 16)
                    for c in range(KC):
                        nc.sync.dma_start_transpose(
                            out=fT[c][cg][:, off:off + 128],
                            in_=b[:, g, 128 * c:128 * c + 128])

            nc.vector.tensor_copy(out=g_sb, in_=g_ps)

        # bounce g through DRAM to get it transposed into [d, c] layout
        g_dram = dramp.tile([2, D], bf16)
        nc.sync.dma_start(out=g_dram, in_=g_sb)
        for c in range(2):
            nc.sync.dma_start(
                out=gT_sb[:, :, c],
                in_=g_dram[c].rearrange("(k p) -> p k", p=128))

        # main pass: similarity block matmuls + exp with fused row-sums
        with tc.tile_pool(name="mps", bufs=2, space="PSUM") as mpp:
            for rb in range(RT):
                zac = zp.tile([128, NCG], f32, tag="zac")
                for cg in range(NCG):
                    ps = mpp.tile([128, CG], f32, tag="ps")
                    for ct in range(CG // 512):
                        for kc in range(KC):
                            nc.tensor.matmul(
                                ps[:, 512 * ct:512 * ct + 512],
                                lhsT=fTr[kc][:, 128 * rb:128 * rb + 128],
                                rhs=fT[kc][cg][:, 512 * ct:512 * ct + 512],
                                start=(kc == 0), stop=(kc == KC - 1))
                    esc = expp.tile([128, CG], bf16, tag="esc")
                    nc.scalar.activation(out=esc, in_=ps, func=A.Exp,
                                         scale=TEMP_INV,
                                         accum_out=zac[:, cg:cg + 1])
                nc.vector.tensor_reduce(out=ZE[:, rb:rb + 1], in_=zac,
                                        axis=X, op=Alu.add)

        # rowdot: rd[i, c] = f_i . g_c for this core's rows
        with tc.tile_pool(name="rps", bufs=2, space="PSUM") as rpp:
            for rb in range(RT):
                rd = rpp.tile([128, 2], f32, tag="rd")
                for kc in range(KC):
                    nc.tensor.matmul(rd, lhsT=fTr[kc][:, 128 * rb:128 * rb + 128],
                                     rhs=gT_sb[:, kc], start=(kc == 0),
                                     stop=(kc == KC - 1))
                nc.vector.tensor_copy(out=rd_sb[:, rb], in_=rd)

        # assembly: per_row = (npos*ln(Z) - 10*rd_sel + 10) * invn
        dexp = ssp.tile([128, RT], f32, tag="dexp")
        nc.scalar.activation(out=dexp, in_=diag_ss, func=A.Exp, scale=TEMP_INV)
        Z = ssp.tile([128, RT], f32, tag="Z")
        nc.vector.tensor_tensor(out=Z, in0=ZE, in1=dexp, op=Alu.subtract)
        lnZ = ssp.tile([128, RT], f32, tag="lnZ")
        nc.scalar.activation(out=lnZ, in_=Z, func=A.Ln)

        sel = ssp.tile([128, RT, 2], f32, tag="sel")
        nc.vector.tensor_tensor(out=sel, in0=rd_sb, in1=ohr_sb, op=Alu.mult)
        rd_sel = ssp.tile([128, RT], f32, tag="rdsel")
        nc.vector.tensor_reduce(out=rd_sel, in_=sel, axis=X, op=Alu.add)

        t1 = ssp.tile([128, RT], f32, tag="t1")
        nc.vector.tensor_tensor(out=t1, in0=npos_sb, in1=lnZ, op=Alu.mult)
        t2 = ssp.tile([128, RT], f32, tag="t2")
        nc.vector.tensor_scalar(out=t2, in0=rd_sel, scalar1=-TEMP_INV,
                                scalar2=TEMP_INV, op0=Alu.mult, op1=Alu.add)
        t3 = ssp.tile([128, RT], f32, tag="t3")
        nc.vector.tensor_tensor(out=t3, in0=t1, in1=t2, op=Alu.add)
        pr = ssp.tile([128, RT], f32, tag="pr")
        nc.vector.tensor_tensor(out=pr, in0=t3, in1=invn_sb, op=Alu.mult)
        nc.sync.dma_start(out=outp.ap(), in_=pr)

    nc.compile()
    return nc


def _get_program():
    global _prog_cache
    if _prog_cache is None:
        _prog_cache = _build_program()
    return _prog_cache


def _prep_inputs(features, labels):
    bf16 = ml_dtypes.bfloat16
    f = np.ascontiguousarray(np.asarray(features, dtype=np.float32)).astype(bf16)
    lab = np.asarray(labels).astype(np.int64)
    oh = np.stack([lab == 0, lab == 1], axis=1)
    ohc = oh.astype(bf16)
    counts = oh.sum(axis=0)
    npos_full = (counts[lab] - 1).astype(np.float32)
    invn_full = (1.0 / (npos_full + EPS)).astype(np.float32)

    in_maps = []
    for k in range(NCORES):
        sl = slice(k * RPC, (k + 1) * RPC)
        in_maps.append({
            "feat": f,
            "xrow": f[sl],
            "ohc": ohc,
            "ohr": np.ascontiguousarray(
                oh[sl].reshape(RT, 128, 2).transpose(1, 0, 2)).astype(np.float32),
            "npos": np.ascontiguousarray(npos_full[sl].reshape(RT, 128).T),
            "invn": np.ascontiguousarray(invn_full[sl].reshape(RT, 128).T),
        })
    return in_maps


def _run(inputs, trace=False, trace_kwargs=None):
    from concourse.bass_utils import run_bass_kernel_spmd

    nc = _get_program()
    in_maps = _prep_inputs(inputs["features"], inputs["labels"])
    res = run_bass_kernel_spmd(nc, in_maps, core_ids=list(range(NCORES)),
                               trace=trace, **(trace_kwargs or {}))
    per_row = np.empty((N,), np.float32)
    for k in range(NCORES):
        # out[p, t] is the loss of global row k*RPC + t*128 + p
        per_row[k * RPC:(k + 1) * RPC] = res.results[k]["out"].T.reshape(RPC)
    loss = np.float32(per_row.mean(dtype=np.float64))
    return loss, res


def kernel(**inputs) -> np.ndarray:
    loss, _ = _run(inputs, trace=False)
    return np.asarray(loss, dtype=np.float32)


# revision 10
# speedup vs baseline: 1.9641x; 1.9641x over previous
"""ContrastiveLoss Trainium2 kernel (8 NeuronCores, SPMD row-sharded).

Math (reference):
    f = features / ||features||_row            (L2 normalize)
    s_ij = (f_i . f_j) / T,  T = 0.1
    Z_i = sum_{j != i} exp(s_ij)
    per_row_i = (num_pos_i * log(Z_i) - sum_j mask_ij s_ij) / (num_pos_i + eps)
    loss = mean(per_row)
where mask = same-label excluding self.  sum_j mask_ij s_ij = 10*(f_i . g_{label_i}) - 10
with g_c = sum_{j: label_j = c} f_j  (class sums) -- avoids any O(N^2) masked work.

Each core owns 1024 rows: computes its (1024 x 8192) similarity block in bf16 on
the PE, exponentiates on ACT with fused row-sum accumulation, and assembles its
per-row losses.  Host only shards/concatenates and takes the final mean.
"""

import numpy as np
import ml_dtypes

TEMP_INV = 10.0  # 1/temperature
EPS = 1e-8
N, D, NCORES = 8192, 512, 8
RPC = N // NCORES        # 1024 rows per core
RT = RPC // 128          # 8 row tiles (128 rows) per core
MT = N // RPC            # 8 column mega-tiles of 1024 rows
CG = 2048                # psum/exp column-group width
NCG = N // CG            # 4 column groups
KC = D // 128            # 4 contraction chunks

_prog_cache = None


def _build_program():
    import concourse.bacc as bacc
    import concourse.tile as tile
    import concourse.hw_specs as hw_specs
    from concourse import mybir

    # Pin every ACT function we use (Exp/Ln/Copy/Identity) to the single table
    # set that contains them all, so walrus never inserts a mid-kernel ~2.7us
    # table switch.  Mutates the functools.cache'd dict in place; indices into
    # act_info.json are preserved because only set *contents* change.
    tabs = hw_specs.get_activation_tables("gen3")
    keep = "natural_log_exp_and_others"
    if keep in tabs:
        for name in tabs:
            if name != keep:
                tabs[name] = set()

    f32, bf16 = mybir.dt.float32, mybir.dt.bfloat16
    A = mybir.ActivationFunctionType
    Alu = mybir.AluOpType
    X = mybir.AxisListType.X

    nc = bacc.Bacc("TRN2", target_bir_lowering=False, debug=False,
                   num_devices=NCORES)

    feat = nc.dram_tensor("feat", [N, D], bf16, kind="ExternalInput")
    xrow = nc.dram_tensor("xrow", [RPC, D], bf16, kind="ExternalInput")
    ohc = nc.dram_tensor("ohc", [N, 2], bf16, kind="ExternalInput")
    ohr = nc.dram_tensor("ohr", [128, RT, 2], f32, kind="ExternalInput")
    npos = nc.dram_tensor("npos", [128, RT], f32, kind="ExternalInput")
    invn = nc.dram_tensor("invn", [128, RT], f32, kind="ExternalInput")
    outp = nc.dram_tensor("out", [128, RT], f32, kind="ExternalOutput")

    featv = feat.ap().rearrange("(m g p) d -> m p g d", p=128, g=RPC // 128)
    xrowv = xrow.ap().rearrange("(g p) d -> p g d", p=128)
    ohcv = ohc.ap().rearrange("(t p) c -> p t c", p=128)

    from contextlib import ExitStack

    with tile.TileContext(nc) as tc, ExitStack() as ctx:
        singles = ctx.enter_context(tc.tile_pool(name="singles", bufs=1))
        xin = ctx.enter_context(tc.tile_pool(name="xin", bufs=3))
        bpool = ctx.enter_context(tc.tile_pool(name="bpool", bufs=3))
        scrp = ctx.enter_context(tc.tile_pool(name="scr", bufs=2))
        ssp = ctx.enter_context(tc.tile_pool(name="ss", bufs=6))
        expp = ctx.enter_context(tc.tile_pool(name="expscr", bufs=3))
        zp = ctx.enter_context(tc.tile_pool(name="zac", bufs=3))
        dramp = ctx.enter_context(tc.tile_pool(name="dram", bufs=1, space="DRAM"))

        # persistent transposed features: fT[c][cg] holds d-chunk c of columns
        # [cg*2048, (cg+1)*2048); fTr[c] the same for this core's own rows.
        # fT[cg][p, kc, col]: transposed features, d = kc*128 + p
        fT = [singles.tile([128, KC, CG], bf16, tag=f"fT{g}", name=f"fT{g}")
              for g in range(NCG)]
        fTr = singles.tile([128, KC, RPC], bf16, tag="fTr", name="fTr")

        ohc_sb = singles.tile([128, N // 128, 2], bf16, tag="ohc")
        nc.sync.dma_start(out=ohc_sb, in_=ohcv)
        ohr_sb = singles.tile([128, RT, 2], f32, tag="ohr")
        nc.sync.dma_start(out=ohr_sb, in_=ohr.ap())
        npos_sb = singles.tile([128, RT], f32, tag="npos")
        nc.sync.dma_start(out=npos_sb, in_=npos.ap())
        invn_sb = singles.tile([128, RT], f32, tag="invn")
        nc.sync.dma_start(out=invn_sb, in_=invn.ap())

        diag_ss = singles.tile([128, RT], f32, tag="diag")
        ZE = singles.tile([128, RT], f32, tag="ZE")
        g_sb = singles.tile([2, D], bf16, tag="gsb")
        gT_sb = singles.tile([128, KC, 2], bf16, tag="gT")
        rd_sb = singles.tile([128, RT, 2], f32, tag="rd")

        def prep_megatile(src_ap, own):
            """Load 1024 rows, normalize, cast to bf16; returns B tile."""
            x = xin.tile([128, RPC // 128, D], bf16, tag="xin")
            nc.sync.dma_start(out=x, in_=src_ap)
            ss = ssp.tile([128, RPC // 128], f32, tag="ss")
            scr = scrp.tile([128, RPC // 128, D], bf16, tag="scr")
            for g in range(RPC // 128):
                nc.vector.tensor_tensor(out=scr[:, g], in0=x[:, g],
                                        in1=x[:, g], op=Alu.mult)
                nc.vector.tensor_reduce(out=ss[:, g:g + 1], in_=scr[:, g],
                                        axis=X, op=Alu.add)
            lnb = ssp.tile([128, RPC // 128], f32, tag="lnb")
            nc.scalar.activation(out=lnb, in_=ss, func=A.Ln)
            rn = ssp.tile([128, RPC // 128], f32, tag="rn")
            nc.scalar.activation(out=rn, in_=lnb, func=A.Exp, scale=-0.5)
            b = bpool.tile([128, RPC // 128, D], bf16, tag="b")
            for g in range(RPC // 128):
                nc.vector.tensor_scalar(
                    out=b[:, g], in0=x[:, g], scalar1=rn[:, g:g + 1],
                    scalar2=None, op0=Alu.mult)
            return b

        with tc.tile_pool(name="gps", bufs=1, space="PSUM") as gpp:
            g_ps = gpp.tile([2, D], f32)

            # own rows first (feeds the matmul lhsT)
            b = prep_megatile(xrowv, own=True)
            scrf = scrp.tile([128, RT, D], f32, tag="scrf")
            for g in range(RT):
                nc.vector.tensor_tensor(out=scrf[:, g], in0=b[:, g],
                                        in1=b[:, g], op=Alu.mult)
                nc.vector.tensor_reduce(out=diag_ss[:, g:g + 1],
                                        in_=scrf[:, g], axis=X, op=Alu.add)
                nc.sync.dma_start_transpose(
                    out=fTr[:, :, 128 * g:128 * g + 128], in_=b[:, g])

            # all column mega-tiles: transpose into fT and accumulate class sums
            for m in range(MT):
                b = prep_megatile(featv[m], own=False)
                for g in range(RPC // 128):
                    t = m * (RPC // 128) + g
                    nc.tensor.matmul(g_ps, lhsT=ohc_sb[:, t], rhs=b[:, g],
                                     start=(t == 0), stop=(t == N // 128 - 1))
                    cg, off = t // 16, 128 * (t % 16)
                    nc.sync.dma_start_transpose(
                        out=fT[cg][:, :, off:off + 128], in_=b[:, g])

            nc.vector.tensor_copy(out=g_sb, in_=g_ps)

        # bounce g through DRAM to get it transposed into [d, c] layout
        g_dram = dramp.tile([2, D], bf16)
        nc.sync.dma_start(out=g_dram, in_=g_sb)
        for c in range(2):
            nc.sync.dma_start(
                out=gT_sb[:, :, c],
                in_=g_dram[c].rearrange("(k p) -> p k", p=128))

        # main pass: similarity block matmuls + exp with fused row-sums
        with tc.tile_pool(name="mps", bufs=2, space="PSUM") as mpp:
            for rb in range(RT):
                zac = zp.tile([128, NCG], f32, tag="zac")
                for cg in range(NCG):
                    ps = mpp.tile([128, CG], f32, tag="ps")
                    for ct in range(CG // 512):
                        for kc in range(KC):
                            nc.tensor.matmul(
                                ps[:, 512 * ct:512 * ct + 512],
                                lhsT=fTr[:, kc, 128 * rb:128 * rb + 128],
                                rhs=fT[cg][:, kc, 512 * ct:512 * ct + 512],
                                start=(kc == 0), stop=(kc == KC - 1))
                    esc = expp.tile([128, CG], bf16, tag="esc")
                    nc.scalar.activation(out=esc, in_=ps, func=A.Exp,
                                         scale=TEMP_INV,
                                         accum_out=zac[:, cg:cg + 1])
                nc.vector.tensor_reduce(out=ZE[:, rb:rb + 1], in_=zac,
                                        axis=X, op=Alu.add)

        # rowdot: rd[i, c] = f_i . g_c for this core's rows
        with tc.tile_pool(name="rps", bufs=2, space="PSUM") as rpp:
            for rb in range(RT):
                rd = rpp.tile([128, 2], f32, tag="rd")
                for kc in range(KC):
                    nc.tensor.matmul(rd, lhsT=fTr[:, kc, 128 * rb:128 * rb + 128],
                                     rhs=gT_sb[:, kc], start=(kc == 0),
                                     stop=(kc == KC - 1))
                nc.vector.tensor_copy(out=rd_sb[:, rb], in_=rd)

        # assembly: per_row = (npos*ln(Z) - 10*rd_sel + 10) * invn
        dexp = ssp.tile([128, RT], f32, tag="dexp")
        nc.scalar.activation(out=dexp, in_=diag_ss, func=A.Exp, scale=TEMP_INV)
        Z = ssp.tile([128, RT], f32, tag="Z")
        nc.vector.tensor_tensor(out=Z, in0=ZE, in1=dexp, op=Alu.subtract)
        lnZ = ssp.tile([128, RT], f32, tag="lnZ")
        nc.scalar.activation(out=lnZ, in_=Z, func=A.Ln)

        sel = ssp.tile([128, RT, 2], f32, tag="sel")
        nc.vector.tensor_tensor(out=sel, in0=rd_sb, in1=ohr_sb, op=Alu.mult)
        rd_sel = ssp.tile([128, RT], f32, tag="rdsel")
        nc.vector.tensor_reduce(out=rd_sel, in_=sel, axis=X, op=Alu.add)

        t1 = ssp.tile([128, RT], f32, tag="t1")
        nc.vector.tensor_tensor(out=t1, in0=npos_sb, in1=lnZ, op=Alu.mult)
        t2 = ssp.tile([128, RT], f32, tag="t2")
        nc.vector.tensor_scalar(out=t2, in0=rd_sel, scalar1=-TEMP_INV,
                                scalar2=TEMP_INV, op0=Alu.mult, op1=Alu.add)
        t3 = ssp.tile([128, RT], f32, tag="t3")
        nc.vector.tensor_tensor(out=t3, in0=t1, in1=t2, op=Alu.add)
        pr = ssp.tile([128, RT], f32, tag="pr")
        nc.vector.tensor_tensor(out=pr, in0=t3, in1=invn_sb, op=Alu.mult)
        nc.sync.dma_start(out=outp.ap(), in_=pr)

    nc.compile()
    return nc


def _get_program():
    global _prog_cache
    if _prog_cache is None:
        _prog_cache = _build_program()
    return _prog_cache


def _prep_inputs(features, labels):
    bf16 = ml_dtypes.bfloat16
    f = np.ascontiguousarray(np.asarray(features, dtype=np.float32)).astype(bf16)
    lab = np.asarray(labels).astype(np.int64)
    oh = np.stack([lab == 0, lab == 1], axis=1)
    ohc = oh.astype(bf16)
    counts = oh.sum(axis=0)
    npos_full = (counts[lab] - 1).astype(np.float32)
    invn_full = (1.0 / (npos_full + EPS)).astype(np.float32)

    in_maps = []
    for k in range(NCORES):
        sl = slice(k * RPC, (k + 1) * RPC)
        in_maps.append({
            "feat": f,
            "xrow": f[sl],
            "ohc": ohc,
            "ohr": np.ascontiguousarray(
                oh[sl].reshape(RT, 128, 2).transpose(1, 0, 2)).astype(np.float32),
            "npos": np.ascontiguousarray(npos_full[sl].reshape(RT, 128).T),
            "invn": np.ascontiguousarray(invn_full[sl].reshape(RT, 128).T),
        })
    return in_maps


def _run(inputs, trace=False, trace_kwargs=None):
    from concourse.bass_utils import run_bass_kernel_spmd

    nc = _get_program()
    in_maps = _prep_inputs(inputs["features"], inputs["labels"])
    res = run_bass_kernel_spmd(nc, in_maps, core_ids=list(range(NCORES)),
                               trace=trace, **(trace_kwargs or {}))
    per_row = np.empty((N,), np.float32)
    for k in range(NCORES):
        # out[p, t] is the loss of global row k*RPC + t*128 + p
        per_row[k * RPC:(k + 1) * RPC] = res.results[k]["out"].T.reshape(RPC)
    loss = np.float32(per_row.mean(dtype=np.float64))
    return loss, res


def kernel(**inputs) -> np.ndarray:
    loss, _ = _run(inputs, trace=False)
    return np.asarray(loss, dtype=np.float32)
